# revision 20
# baseline (speedup 1.0000x reference)
"""Trainium2 Bass kernel for chunked causal linear attention (elu+1 feature map).

Reference computation (B=2, N=2048, D=1024, DHAT=512, H=16, F=32, G=64):
    Q = phi(x @ wq + bq), K = phi(x @ wk + bk), V = x @ wv + bv   (per-head split)
    kv_t = cumsum_t(K_t outer V_t);  Z_t = 1/(Q_t . cumsum_t(K)_t + 1e-6)
    out_t = (Q_t . kv_t) * Z_t;  y = out @ wo + bo
with phi(u) = elu(u) + 1 = min(exp(u), max(u + 1, 1)).

Sharding over 8 cores: core c handles batch b = c//4 and heads 4r..4r+3
(r = c%4).  Each core projects its head slice, runs chunk-parallel linear
attention (chunk C=256: intra-chunk masked QK^T + inter-chunk running state
S[f,g]), and computes a PARTIAL output projection through its 256 rows of
wo.  The host sums the 4 partials per batch plus bo (the output-projection
all-reduce realized at unshard time) — on-device collectives on this setup
cost ~25x their table values, far more than the whole compute.

Matmul operands are bf16 (fp32 PSUM accumulation; fp32 state & denominator
path).  V/K chunk transposes ride the idle DMA engines (XBAR transpose),
as does the denominator partition-broadcast.
"""
import os
import sys
import types

sys.path.insert(0, "/opt/trn_rl_repo")

import ml_dtypes
import numpy as np

# ---- problem constants (hardcoded; kernel.py must be self-contained) ----
B, N, D, DHAT, H = 2, 2048, 1024, 512, 16
F = DHAT // H        # 32
G = D // H           # 64
NCORES = 8
CHUNK = 256          # attention chunk along t
NCHUNK = N // CHUNK  # 8
JLOC = 4 * G         # 256 local attention features per core
TB = 512             # projection t-block
BF16NP = ml_dtypes.bfloat16


def _install_ntff_hook():
    """Register the axon NTFF profiling hook (stub antenv lacks axon_hooks)."""
    if "antenv.axon_hooks" in sys.modules:
        return
    try:
        from trn_agent_boot.trn_boot import _ntff_profile_via_ctypes
        hook = _ntff_profile_via_ctypes("/opt/axon/libaxon_pjrt.so")
    except Exception:
        hook = None
    m = types.ModuleType("antenv.axon_hooks")
    m.get_axon_ntff_profile_hook = lambda: hook
    m.set_axon_ntff_profile_hook = lambda h: None
    sys.modules["antenv.axon_hooks"] = m


def build_nc():
    import concourse.bass as bass
    import concourse.mybir as mybir
    import concourse.tile as tile
    from concourse import bacc

    F32 = mybir.dt.float32
    BF16 = mybir.dt.bfloat16
    AF = mybir.ActivationFunctionType
    ALU = mybir.AluOpType

    nc = bacc.Bacc("TRN2", target_bir_lowering=False, debug=False,
                   num_devices=NCORES)

    # ---- per-core DRAM parameters (bf16 operands cast on host) ----
    xT_e = nc.declare_dram_parameter("xT", [D, N], BF16, isOutput=False)
    wq_e = nc.declare_dram_parameter("wq", [D, 4 * F], BF16, isOutput=False)
    wk_e = nc.declare_dram_parameter("wk", [D, 4 * F], BF16, isOutput=False)
    wv_e = nc.declare_dram_parameter("wv", [D, JLOC], BF16, isOutput=False)
    wo_e = nc.declare_dram_parameter("wo", [JLOC, D], BF16, isOutput=False)
    bq_e = nc.declare_dram_parameter("bq", [4 * F, 1], F32, isOutput=False)
    bk_e = nc.declare_dram_parameter("bk", [4 * F, 1], F32, isOutput=False)
    bv_e = nc.declare_dram_parameter("bv", [JLOC, 1], F32, isOutput=False)
    y_e = nc.declare_dram_parameter("y", [N, D], F32, isOutput=True)
    dbg = os.environ.get("KDEBUG")
    if dbg:
        dq_e = nc.declare_dram_parameter("dbg_q", [F, N], F32, isOutput=True)
        da_e = nc.declare_dram_parameter("dbg_a", [128, N], F32, isOutput=True)

    # causal masks for the two 128-row s-blocks of a 256-wide chunk
    m0 = np.zeros((128, CHUNK), np.float32)
    m1 = np.zeros((128, CHUNK), np.float32)
    for s in range(128):
        m0[s, s:] = 1.0
        m1[s, 128 + s:] = 1.0
    mask0_d = nc.inline_tensor(m0.astype(BF16NP), "mask0")
    mask1_d = nc.inline_tensor(m1.astype(BF16NP), "mask1")
    va0 = np.zeros((128, 128), np.float32)
    va0[:, G] = 1.0
    vainit_d = nc.inline_tensor(va0.astype(BF16NP), "vainitc")
    zeros_d = nc.inline_tensor(np.zeros((128, 128), BF16NP), "zerosc")

    with tile.TileContext(nc) as tc:
        with (
            tc.tile_pool(name="persist", bufs=1) as pers,
            tc.tile_pool(name="xin", bufs=8) as xin,
            tc.tile_pool(name="ppool", bufs=2, space="PSUM") as pp,
            tc.tile_pool(name="apool", bufs=3, space="PSUM") as apsum,
            tc.tile_pool(name="spool", bufs=1, space="PSUM") as spsum,
            tc.tile_pool(name="opool", bufs=2, space="PSUM") as opsum,
            tc.tile_pool(name="work", bufs=4) as work,
            tc.tile_pool(name="ssb", bufs=2) as ssb,
            tc.tile_pool(name="ysb", bufs=3) as ysb,
            tc.tile_pool(name="dend", bufs=4, space="DRAM") as dend,
        ):
            # ---- persistent SBUF state ----
            mask_sb = [pers.tile([128, CHUNK], BF16, tag=f"mask{i}",
                                 name=f"mask{i}") for i in range(2)]
            nc.sync.dma_start(mask_sb[0][:], mask0_d[:])
            nc.sync.dma_start(mask_sb[1][:], mask1_d[:])

            KD = D // 128  # 8 contraction tiles
            wq_sb = [pers.tile([128, 4 * F], BF16, tag=f"wq{k}", name=f"wq{k}")
                     for k in range(KD)]
            wk_sb = [pers.tile([128, 4 * F], BF16, tag=f"wk{k}", name=f"wk{k}")
                     for k in range(KD)]
            for k in range(KD):
                nc.sync.dma_start(wq_sb[k][:], wq_e[128 * k:128 * (k + 1), :])
                nc.sync.dma_start(wk_sb[k][:], wk_e[128 * k:128 * (k + 1), :])
            wv_sb = [[pers.tile([128, 128], BF16, tag=f"wv{k}_{m}",
                                name=f"wv{k}_{m}") for m in range(2)]
                     for k in range(KD)]
            for k in range(KD):
                for m in range(2):
                    nc.sync.dma_start(
                        wv_sb[k][m][:],
                        wv_e[128 * k:128 * (k + 1), 128 * m:128 * (m + 1)])
            wo_sb = [pers.tile([128, D], BF16, tag=f"wo{j}", name=f"wo{j}")
                     for j in range(2)]
            for j in range(2):
                nc.sync.dma_start(wo_sb[j][:], wo_e[128 * j:128 * (j + 1), :])

            bq_sb = pers.tile([4 * F, 1], F32)
            bk_sb = pers.tile([4 * F, 1], F32)
            bv_sb = [pers.tile([128, 1], F32, tag=f"bv{i}", name=f"bv{i}")
                     for i in range(2)]
            nc.sync.dma_start(bq_sb[:], bq_e[:])
            nc.sync.dma_start(bk_sb[:], bk_e[:])
            for i in range(2):
                nc.sync.dma_start(bv_sb[i][:], bv_e[128 * i:128 * (i + 1), :])
            # bias+1 variants for the relu(u)+1 branch of phi
            bq1_sb = pers.tile([4 * F, 1], F32)
            bk1_sb = pers.tile([4 * F, 1], F32)
            nc.vector.tensor_scalar(bq1_sb[:], bq_sb[:], 1.0, None, op0=ALU.add)
            nc.vector.tensor_scalar(bk1_sb[:], bk_sb[:], 1.0, None, op0=ALU.add)

            # per-head feature-major tiles (base partition 0 for PE operands)
            qh = [pers.tile([F, N], BF16, tag=f"qh{h}", name=f"qh{h}")
                  for h in range(4)]
            kh = [pers.tile([F, N], BF16, tag=f"kh{h}", name=f"kh{h}")
                  for h in range(4)]
            vh = [pers.tile([G, N], BF16, tag=f"vh{h}", name=f"vh{h}")
                  for h in range(4)]
            aT = [pers.tile([128, N], BF16, tag=f"aT{i}", name=f"aT{i}")
                  for i in range(2)]
            # t-major V_aug / K tiles, padded to [128,128]; pads initialized
            # once (ones column for V_aug baked into init)
            va_p = [[pers.tile([128, 128], BF16, tag=f"va{h}_{s}",
                               name=f"va{h}_{s}") for s in range(2)]
                    for h in range(4)]
            ktm_p = [[pers.tile([128, 128], BF16, tag=f"ktm{h}_{s}",
                                name=f"ktm{h}_{s}") for s in range(2)]
                     for h in range(4)]
            for h in range(4):
                for s in range(2):
                    nc.sync.dma_start(va_p[h][s][:], vainit_d[:])
                    nc.sync.dma_start(ktm_p[h][s][:], zeros_d[:])

            # ---- projections, feature-major, per t-block ----
            for tb in range(N // TB):
                tsl = slice(TB * tb, TB * (tb + 1))
                xt = [xin.tile([128, TB], BF16, tag="xt", name="xt")
                      for _ in range(KD)]
                for k in range(KD):
                    nc.sync.dma_start(xt[k][:], xT_e[128 * k:128 * (k + 1), tsl])

                for (w_sb, b_sb, b1_sb, dsts) in (
                        (wq_sb, bq_sb, bq1_sb, qh), (wk_sb, bk_sb, bk1_sb, kh)):
                    ps = pp.tile([128, TB], F32, tag="proj", name="proj_ps")
                    for k in range(KD):
                        nc.tensor.matmul(ps[:], w_sb[k][:], xt[k][:],
                                         start=(k == 0), stop=(k == KD - 1))
                    # phi(u) = min(exp(u), max(u + 1, 1)), bias folded in
                    e_sb = work.tile([128, TB], BF16, tag="phi_e", name="phi_e")
                    nc.scalar.activation(e_sb[:], ps[:], AF.Exp, bias=b_sb[:])
                    u_sb = work.tile([128, TB], BF16, tag="phi_u", name="phi_u")
                    nc.vector.tensor_scalar(u_sb[:], ps[:], b1_sb[:], 1.0,
                                            op0=ALU.add, op1=ALU.max)
                    for h in range(4):
                        fr = slice(F * h, F * (h + 1))
                        nc.vector.tensor_tensor(dsts[h][:, tsl], e_sb[fr, :],
                                                u_sb[fr, :], op=ALU.min)

                for m in range(2):
                    ps = pp.tile([128, TB], F32, tag="proj", name="proj_ps")
                    for k in range(KD):
                        nc.tensor.matmul(ps[:], wv_sb[k][m][:], xt[k][:],
                                         start=(k == 0), stop=(k == KD - 1))
                    for half in range(2):
                        h = 2 * m + half
                        gr = slice(G * half, G * (half + 1))
                        nc.scalar.activation(
                            vh[h][:, tsl], ps[gr, :], AF.Identity,
                            bias=bv_sb[m][gr, :])

            # ---- chunked linear attention, per head ----
            for h in range(4):
                atile = aT[h // 2]
                vrow = 64 * (h % 2)
                s_prev = None      # fp32 running state (SBUF)
                s_bf = None        # bf16 copy for the PE
                for i in range(NCHUNK):
                    t0 = CHUNK * i
                    csl = slice(t0, t0 + CHUNK)
                    # --- intra-chunk A^T = K Q^T per 128-row s-block ---
                    am = []
                    for sb2 in range(2):
                        ssl = slice(t0 + 128 * sb2, t0 + 128 * (sb2 + 1))
                        a_ps = apsum.tile([128, CHUNK], F32, tag="A",
                                          name="a_ps")
                        nc.tensor.matmul(a_ps[:], kh[h][:, ssl], qh[h][:, csl],
                                         start=True, stop=True)
                        am_sb = work.tile([128, CHUNK], BF16, tag="am",
                                          name="am")
                        nc.vector.tensor_tensor(am_sb[:], a_ps[:],
                                                mask_sb[sb2][:], op=ALU.mult)
                        am.append(am_sb)
                        # t-major V_aug and K via DMA XBAR transpose
                        nc.sync.dma_start_transpose(
                            va_p[h][sb2][:, 0:G], vh[h][:, ssl])
                        nc.sync.dma_start_transpose(
                            ktm_p[h][sb2][:, 0:F], kh[h][:, ssl])
                    vaug = [va_p[h][0], va_p[h][1]]
                    ktm = [ktm_p[h][0], ktm_p[h][1]]
                    # --- out^T (rows 0..G-1) + denominator (row G) ---
                    o_ps = opsum.tile([128, CHUNK], F32, tag="o", name="o_ps")
                    nc.tensor.matmul(o_ps[:], vaug[0][:], am[0][:],
                                     start=True, stop=False)
                    nc.tensor.matmul(o_ps[:], vaug[1][:], am[1][:],
                                     start=False, stop=(s_bf is None))
                    if s_bf is not None:
                        nc.tensor.matmul(o_ps[:], s_bf[:], qh[h][:, csl],
                                         start=False, stop=True)
                    # --- state update S += K_chunk^T V_aug (fp32 in SBUF) ---
                    if i < NCHUNK - 1:
                        s_ps = spsum.tile([128, 128], F32, tag="S", name="s_ps")
                        nc.tensor.matmul(s_ps[:], ktm[0][:], vaug[0][:],
                                         start=True, stop=False)
                        nc.tensor.matmul(s_ps[:], ktm[1][:], vaug[1][:],
                                         start=False, stop=True)
                        s_sb = ssb.tile([F, 128], F32, tag="ssb", name="s_sb")
                        if s_prev is None:
                            nc.vector.tensor_copy(s_sb[:], s_ps[0:F, :])
                        else:
                            nc.vector.tensor_tensor(s_sb[:], s_ps[0:F, :],
                                                    s_prev[:], op=ALU.add)
                        s_prev = s_sb
                        s_bf = ssb.tile([F, 128], BF16, tag="sbf", name="s_bf")
                        nc.vector.tensor_copy(s_bf[:], s_sb[:])
                    # --- normalize: attn^T = out^T / denom (fp32 path) ---
                    den_sb = work.tile([1, CHUNK], F32, tag="den", name="den")
                    nc.vector.tensor_scalar(den_sb[:], o_ps[G:G + 1, :],
                                            1e-6, None, op0=ALU.add)
                    den_dram = dend.tile([1, CHUNK], F32, name="den_dram")
                    nc.sync.dma_start(den_dram[:], den_sb[:])
                    dbc = work.tile([G, CHUNK], F32, tag="dbc", name="dbc")
                    den_bcast = bass.AP(tensor=den_dram.tensor,
                                        offset=den_dram.offset,
                                        ap=[[0, G], [1, CHUNK]])
                    nc.sync.dma_start(dbc[:], den_bcast)
                    rec_sb = work.tile([G, CHUNK], F32, tag="rec", name="rec")
                    scr = work.tile([G, CHUNK], F32, tag="scr", name="scr")
                    nc.vector.reciprocal_approx_accurate(rec_sb[:], dbc[:],
                                                         scr[:])
                    nc.vector.tensor_tensor(
                        atile[vrow:vrow + G, csl], o_ps[0:G, :], rec_sb[:],
                        op=ALU.mult)

            if dbg:
                nc.sync.dma_start(dq_e[:], qh[0][:].bitcast(F32))
                nc.sync.dma_start(da_e[:], aT[0][:].bitcast(F32))

            # ---- partial output projection: y[t, e] = attn^T.T @ wo ----
            for tt in range(N // 128):
                tsl = slice(128 * tt, 128 * (tt + 1))
                for eb in range(2):
                    esl = slice(512 * eb, 512 * (eb + 1))
                    y_ps = opsum.tile([128, 512], F32, tag="o", name="y_ps")
                    for j in range(2):
                        nc.tensor.matmul(y_ps[:], aT[j][:, tsl],
                                         wo_sb[j][:, esl],
                                         start=(j == 0), stop=(j == 1))
                    y_sb = ysb.tile([128, 512], F32, tag="ysb", name="y_sb")
                    nc.scalar.activation(y_sb[:], y_ps[:], AF.Copy)
                    nc.sync.dma_start(y_e[tsl, esl], y_sb[:])

    nc.compile()
    return nc


def make_in_maps(x, wq, bq, wk, bk, wv, bv, wo, bo):
    x = np.asarray(x, np.float32)
    in_maps = []
    for c in range(NCORES):
        b, r = divmod(c, 4)
        in_maps.append({
            "xT": np.ascontiguousarray(x[b].T).astype(BF16NP),
            "wq": np.ascontiguousarray(
                np.asarray(wq)[:, 128 * r:128 * (r + 1)]).astype(BF16NP),
            "wk": np.ascontiguousarray(
                np.asarray(wk)[:, 128 * r:128 * (r + 1)]).astype(BF16NP),
            "wv": np.ascontiguousarray(
                np.asarray(wv)[:, 256 * r:256 * (r + 1)]).astype(BF16NP),
            "wo": np.ascontiguousarray(
                np.asarray(wo)[256 * r:256 * (r + 1), :]).astype(BF16NP),
            "bq": np.ascontiguousarray(
                np.asarray(bq)[128 * r:128 * (r + 1)],
                dtype=np.float32).reshape(-1, 1),
            "bk": np.ascontiguousarray(
                np.asarray(bk)[128 * r:128 * (r + 1)],
                dtype=np.float32).reshape(-1, 1),
            "bv": np.ascontiguousarray(
                np.asarray(bv)[256 * r:256 * (r + 1)],
                dtype=np.float32).reshape(-1, 1),
        })
    return in_maps


def assemble(results, bo):
    y = np.zeros((B, N, D), np.float32)
    for c in range(NCORES):
        y[c // 4] += results[c]["y"]
    return y + np.asarray(bo, np.float32).reshape(1, 1, D)


_NC_CACHE = {}


def run(inputs, trace=False):
    _install_ntff_hook()
    from concourse.bass_utils import run_bass_kernel_spmd
    if "nc" not in _NC_CACHE:
        _NC_CACHE["nc"] = build_nc()
    nc = _NC_CACHE["nc"]
    in_maps = make_in_maps(**inputs)
    res = run_bass_kernel_spmd(nc, in_maps, core_ids=list(range(NCORES)),
                               trace=trace)
    return assemble(res.results, inputs["bo"]), res.exec_time_ns


def kernel(**inputs) -> np.ndarray:
    y, _ = run(inputs, trace=False)
    return y


# revision 25
# speedup vs baseline: 1.8617x; 1.8617x over previous
"""Trainium2 Bass kernel for chunked causal linear attention (elu+1 feature map).

Reference computation (B=2, N=2048, D=1024, DHAT=512, H=16, F=32, G=64):
    Q = phi(x @ wq + bq), K = phi(x @ wk + bk), V = x @ wv + bv   (per-head split)
    kv_t = cumsum_t(K_t outer V_t);  Z_t = 1/(Q_t . cumsum_t(K)_t + 1e-6)
    out_t = (Q_t . kv_t) * Z_t;  y = out @ wo + bo
with phi(u) = elu(u) + 1 = min(exp(u), max(u + 1, 1)).

Sharding over 8 cores: core c handles batch b = c//4 and heads 4r..4r+3
(r = c%4).  Each core projects its head slice, runs chunk-parallel linear
attention (chunk C=256: intra-chunk masked QK^T + inter-chunk running state
S[f,g]), and computes a PARTIAL output projection through its 256 rows of
wo.  The host sums the 4 partials per batch plus bo (the output-projection
all-reduce realized at unshard time) — on-device collectives on this setup
cost ~25x their table values, far more than the whole compute.

Matmul operands are bf16 (fp32 PSUM accumulation; fp32 state & denominator
path).  V/K chunk transposes ride the idle DMA engines (XBAR transpose),
as does the denominator partition-broadcast.
"""
import os
import sys
import types

sys.path.insert(0, "/opt/trn_rl_repo")

import ml_dtypes
import numpy as np

# ---- problem constants (hardcoded; kernel.py must be self-contained) ----
B, N, D, DHAT, H = 2, 2048, 1024, 512, 16
F = DHAT // H        # 32
G = D // H           # 64
NCORES = 8
CHUNK = 256          # attention chunk along t
NCHUNK = N // CHUNK  # 8
JLOC = 4 * G         # 256 local attention features per core
TB = 512             # projection t-block
BF16NP = ml_dtypes.bfloat16


def _install_ntff_hook():
    """Register the axon NTFF profiling hook (stub antenv lacks axon_hooks)."""
    if "antenv.axon_hooks" in sys.modules:
        return
    try:
        from trn_agent_boot.trn_boot import _ntff_profile_via_ctypes
        hook = _ntff_profile_via_ctypes("/opt/axon/libaxon_pjrt.so")
    except Exception:
        hook = None
    m = types.ModuleType("antenv.axon_hooks")
    m.get_axon_ntff_profile_hook = lambda: hook
    m.set_axon_ntff_profile_hook = lambda h: None
    sys.modules["antenv.axon_hooks"] = m


def build_nc():
    import concourse.bass as bass
    import concourse.mybir as mybir
    import concourse.tile as tile
    from concourse import bacc

    F32 = mybir.dt.float32
    BF16 = mybir.dt.bfloat16
    AF = mybir.ActivationFunctionType
    ALU = mybir.AluOpType

    nc = bacc.Bacc("TRN2", target_bir_lowering=False, debug=False,
                   num_devices=NCORES)

    # ---- per-core DRAM parameters (bf16 operands cast on host) ----
    xT_e = nc.declare_dram_parameter("xT", [D, N], BF16, isOutput=False)
    wq_e = nc.declare_dram_parameter("wq", [D, 4 * F], BF16, isOutput=False)
    wk_e = nc.declare_dram_parameter("wk", [D, 4 * F], BF16, isOutput=False)
    wv_e = nc.declare_dram_parameter("wv", [D, JLOC], BF16, isOutput=False)
    wo_e = nc.declare_dram_parameter("wo", [JLOC, D], BF16, isOutput=False)
    bq_e = nc.declare_dram_parameter("bq", [4 * F, 1], F32, isOutput=False)
    bk_e = nc.declare_dram_parameter("bk", [4 * F, 1], F32, isOutput=False)
    bv_e = nc.declare_dram_parameter("bv", [JLOC, 1], F32, isOutput=False)
    y_e = nc.declare_dram_parameter("y", [N, D], F32, isOutput=True)
    dbg = os.environ.get("KDEBUG")
    if dbg:
        dq_e = nc.declare_dram_parameter("dbg_q", [F, N], F32, isOutput=True)
        da_e = nc.declare_dram_parameter("dbg_a", [128, N], F32, isOutput=True)

    # causal masks for the two 128-row s-blocks of a 256-wide chunk
    m0 = np.zeros((128, CHUNK), np.float32)
    m1 = np.zeros((128, CHUNK), np.float32)
    for s in range(128):
        m0[s, s:] = 1.0
        m1[s, 128 + s:] = 1.0
    maskc_d = nc.inline_tensor(
        np.concatenate([m0, m1], axis=1).astype(BF16NP), "maskc")
    va0 = np.zeros((128, 128), np.float32)
    va0[:, 96] = 1.0   # ones column for the denominator row
    vainit_d = nc.inline_tensor(va0.astype(BF16NP), "vainitc")
    ident_d = nc.inline_tensor(np.eye(128, dtype=np.float32).astype(BF16NP),
                               "identc")

    with tile.TileContext(nc) as tc:
        with (
            tc.tile_pool(name="persist", bufs=1) as pers,
            tc.tile_pool(name="xin", bufs=8) as xin,
            tc.tile_pool(name="ppool", bufs=2, space="PSUM") as pp,
            tc.tile_pool(name="apool", bufs=3, space="PSUM") as apsum,
            tc.tile_pool(name="spool", bufs=1, space="PSUM") as spsum,
            tc.tile_pool(name="opool", bufs=2, space="PSUM") as opsum,
            tc.tile_pool(name="dend", bufs=4, space="DRAM") as dend,
            tc.tile_pool(name="work", bufs=4) as work,
            tc.tile_pool(name="ssb", bufs=2) as ssb,
            tc.tile_pool(name="ysb", bufs=3) as ysb,
        ):
            # ---- persistent SBUF state ----
            maskc_sb = pers.tile([128, 2 * CHUNK], BF16)
            nc.sync.dma_start(maskc_sb[:], maskc_d[:])
            ident = pers.tile([128, 128], BF16)
            nc.sync.dma_start(ident[:], ident_d[:])

            KD = D // 128  # 8 contraction tiles
            wq_sb = [pers.tile([128, 4 * F], BF16, tag=f"wq{k}", name=f"wq{k}")
                     for k in range(KD)]
            wk_sb = [pers.tile([128, 4 * F], BF16, tag=f"wk{k}", name=f"wk{k}")
                     for k in range(KD)]
            for k in range(KD):
                nc.sync.dma_start(wq_sb[k][:], wq_e[128 * k:128 * (k + 1), :])
                nc.sync.dma_start(wk_sb[k][:], wk_e[128 * k:128 * (k + 1), :])
            wv_sb = [[pers.tile([128, 128], BF16, tag=f"wv{k}_{m}",
                                name=f"wv{k}_{m}") for m in range(2)]
                     for k in range(KD)]
            for k in range(KD):
                for m in range(2):
                    nc.sync.dma_start(
                        wv_sb[k][m][:],
                        wv_e[128 * k:128 * (k + 1), 128 * m:128 * (m + 1)])
            wo_sb = [pers.tile([128, D], BF16, tag=f"wo{j}", name=f"wo{j}")
                     for j in range(2)]
            for j in range(2):
                nc.sync.dma_start(wo_sb[j][:], wo_e[128 * j:128 * (j + 1), :])

            bq_sb = pers.tile([4 * F, 1], F32)
            bk_sb = pers.tile([4 * F, 1], F32)
            bv_sb = [pers.tile([128, 1], F32, tag=f"bv{i}", name=f"bv{i}")
                     for i in range(2)]
            nc.sync.dma_start(bq_sb[:], bq_e[:])
            nc.sync.dma_start(bk_sb[:], bk_e[:])
            for i in range(2):
                nc.sync.dma_start(bv_sb[i][:], bv_e[128 * i:128 * (i + 1), :])
            # bias+1 variants for the relu(u)+1 branch of phi
            bq1_sb = pers.tile([4 * F, 1], F32)
            bk1_sb = pers.tile([4 * F, 1], F32)
            eps_sb = pers.tile([1, 1], F32)
            nc.vector.memset(eps_sb[:], 1e-6)
            nc.vector.tensor_scalar(bq1_sb[:], bq_sb[:], 1.0, None, op0=ALU.add)
            nc.vector.tensor_scalar(bk1_sb[:], bk_sb[:], 1.0, None, op0=ALU.add)

            # per-head feature-major tiles.  Layout (partition-alignment
            # rules: 64-row reads start at 0/64, 32-row reads at 0/32/64/96):
            #   vkh[h]: rows 0:64 = V^T, rows 64:96 = K^T
            #   qh[h]:  rows 64:96 = Q^T  (base 64 to match K^T for the PE)
            qh = [pers.tile([G + F, N], BF16, tag=f"qh{h}", name=f"qh{h}")
                  for h in range(4)]
            vkh = [pers.tile([G + F, N], BF16, tag=f"vkh{h}", name=f"vkh{h}")
                   for h in range(4)]
            aT = [pers.tile([128, N], BF16, tag=f"aT{i}", name=f"aT{i}")
                  for i in range(2)]
            # t-major [K_t | V_t | ones | zeros] tiles, padded to [128,128];
            # ones column (96) and zero pad initialized once
            vk_p = [[pers.tile([128, 128], BF16, tag=f"vk{h}_{s}",
                               name=f"vk{h}_{s}") for s in range(2)]
                    for h in range(4)]
            for h in range(4):
                for s in range(2):
                    nc.sync.dma_start(vk_p[h][s][:], vainit_d[:])

            # ---- projections, feature-major, per t-block ----
            for tb in range(N // TB):
                tsl = slice(TB * tb, TB * (tb + 1))
                xt = [xin.tile([128, TB], BF16, tag="xt", name="xt")
                      for _ in range(KD)]
                for k in range(KD):
                    nc.sync.dma_start(xt[k][:], xT_e[128 * k:128 * (k + 1), tsl])

                for (w_sb, b_sb, b1_sb, is_q) in (
                        (wq_sb, bq_sb, bq1_sb, True),
                        (wk_sb, bk_sb, bk1_sb, False)):
                    ps = pp.tile([128, TB], F32, tag="proj", name="proj_ps")
                    for k in range(KD):
                        nc.tensor.matmul(ps[:], w_sb[k][:], xt[k][:],
                                         start=(k == 0), stop=(k == KD - 1))
                    # phi(u) = min(exp(u), max(u + 1, 1)), bias folded in
                    e_sb = work.tile([128, TB], BF16, tag="phi_e", name="phi_e")
                    nc.scalar.activation(e_sb[:], ps[:], AF.Exp, bias=b_sb[:])
                    u_sb = work.tile([128, TB], BF16, tag="phi_u", name="phi_u")
                    nc.vector.tensor_scalar(u_sb[:], ps[:], b1_sb[:], 1.0,
                                            op0=ALU.add, op1=ALU.max)
                    for h in range(4):
                        fr = slice(F * h, F * (h + 1))
                        dst = (qh[h] if is_q else vkh[h])[G:G + F, tsl]
                        nc.vector.tensor_tensor(dst, e_sb[fr, :],
                                                u_sb[fr, :], op=ALU.min)

                for m in range(2):
                    ps = pp.tile([128, TB], F32, tag="proj", name="proj_ps")
                    for k in range(KD):
                        nc.tensor.matmul(ps[:], wv_sb[k][m][:], xt[k][:],
                                         start=(k == 0), stop=(k == KD - 1))
                    for half in range(2):
                        h = 2 * m + half
                        gr = slice(G * half, G * (half + 1))
                        nc.scalar.activation(
                            vkh[h][0:G, tsl], ps[gr, :], AF.Identity,
                            bias=bv_sb[m][gr, :])

            # ---- chunked linear attention, per head ----
            # Per chunk/s-block, ONE PE transpose of vkh[:, ssl] yields the
            # t-major [K_t | V_t] block; vk_p's col 96 holds ones so mm2a's
            # output row 96 accumulates the denominator.  o_ps row layout:
            # 0:32 junk (K.Am), 32:96 numerator, 96 denominator.
            for h in range(4):
                atile = aT[h // 2]
                vrow = 64 * (h % 2)
                s_prev = None      # fp32 running state (SBUF)
                s_bf = None        # bf16 copy for the PE
                for i in range(NCHUNK):
                    t0 = CHUNK * i
                    csl = slice(t0, t0 + CHUNK)
                    # intra-chunk A^T (both s-blocks into one psum tile),
                    # plus the combined K/V transpose per s-block
                    a_ps = apsum.tile([128, 2 * CHUNK], F32, tag="A",
                                      name="a_ps")
                    for sb2 in range(2):
                        ssl = slice(t0 + 128 * sb2, t0 + 128 * (sb2 + 1))
                        nc.tensor.matmul(a_ps[:, CHUNK * sb2:CHUNK * (sb2 + 1)],
                                         vkh[h][G:G + F, ssl],
                                         qh[h][G:G + F, csl],
                                         start=True, stop=True)
                        vt_ps = apsum.tile([128, F + G], BF16, tag="A",
                                           name="vt_ps")
                        nc.tensor.transpose(vt_ps[:], vkh[h][:, ssl],
                                            ident[0:F + G, 0:F + G])
                        nc.vector.tensor_copy(vk_p[h][sb2][:, 0:F + G],
                                              vt_ps[:])
                    am_sb = work.tile([128, 2 * CHUNK], BF16, tag="am",
                                      name="am")
                    nc.vector.tensor_tensor(am_sb[:], a_ps[:], maskc_sb[:],
                                            op=ALU.mult)
                    # out^T: rows 0:64 numerator, row 96 denominator
                    o_ps = opsum.tile([128, CHUNK], F32, tag="o", name="o_ps")
                    nc.tensor.matmul(o_ps[:], vk_p[h][0][:],
                                     am_sb[:, 0:CHUNK],
                                     start=True, stop=False)
                    if s_bf is not None:
                        nc.tensor.matmul(o_ps[0:97, :], s_bf[G:G + F, :],
                                         qh[h][G:G + F, csl], start=False,
                                         stop=False)
                    nc.tensor.matmul(o_ps[:], vk_p[h][1][:],
                                     am_sb[:, CHUNK:2 * CHUNK],
                                     start=False, stop=True)
                    # state update S += K_chunk^T [K|V|ones] (fp32 in SBUF)
                    if i < NCHUNK - 1:
                        s_ps = spsum.tile([F, 97], F32, tag="S", name="s_ps")
                        nc.tensor.matmul(s_ps[:], vk_p[h][0][:, G:G + F],
                                         vk_p[h][0][:, 0:97],
                                         start=True, stop=False)
                        nc.tensor.matmul(s_ps[:], vk_p[h][1][:, G:G + F],
                                         vk_p[h][1][:, 0:97],
                                         start=False, stop=True)
                        s_sb = ssb.tile([G + F, 97], F32, tag="ssb",
                                        name="s_sb")
                        if s_prev is None:
                            nc.vector.tensor_copy(s_sb[G:G + F, :], s_ps[:])
                        else:
                            nc.vector.tensor_tensor(s_sb[G:G + F, :], s_ps[:],
                                                    s_prev[G:G + F, :],
                                                    op=ALU.add)
                        s_prev = s_sb
                        s_bf = ssb.tile([G + F, 97], BF16, tag="sbf",
                                        name="s_bf")
                        nc.vector.tensor_copy(s_bf[G:G + F, :],
                                              s_ps[:] if False else
                                              s_sb[G:G + F, :])
                    # normalize: attn^T = num / (den + 1e-6)  (fp32 path)
                    den_sb = work.tile([1, CHUNK], F32, tag="den", name="den")
                    nc.scalar.activation(den_sb[:], o_ps[96:97, :],
                                         AF.Identity, bias=eps_sb[:])
                    den_dram = dend.tile([1, CHUNK], F32, name="den_dram")
                    nc.sync.dma_start(den_dram[:], den_sb[:])
                    dbc = work.tile([G, CHUNK], F32, tag="dbc", name="dbc")
                    den_bcast = bass.AP(tensor=den_dram.tensor,
                                        offset=den_dram.offset,
                                        ap=[[0, G], [1, CHUNK]])
                    nc.sync.dma_start(dbc[:], den_bcast)
                    rec_sb = work.tile([G, CHUNK], F32, tag="rec", name="rec")
                    nc.vector.reciprocal_approx_fast(rec_sb[:], dbc[:])
                    nc.vector.tensor_tensor(
                        atile[vrow:vrow + G, csl], o_ps[0:G, :], rec_sb[:],
                        op=ALU.mult)

            if dbg:
                nc.sync.dma_start(dq_e[:], qh[0][:].bitcast(F32))
                nc.sync.dma_start(da_e[:], aT[0][:].bitcast(F32))

            # ---- partial output projection: y[t, e] = attn^T.T @ wo ----
            for tt in range(N // 128):
                tsl = slice(128 * tt, 128 * (tt + 1))
                for eb in range(2):
                    esl = slice(512 * eb, 512 * (eb + 1))
                    y_ps = opsum.tile([128, 512], F32, tag="o", name="y_ps")
                    for j in range(2):
                        nc.tensor.matmul(y_ps[:], aT[j][:, tsl],
                                         wo_sb[j][:, esl],
                                         start=(j == 0), stop=(j == 1))
                    y_sb = ysb.tile([128, 512], F32, tag="ysb", name="y_sb")
                    nc.scalar.activation(y_sb[:], y_ps[:], AF.Copy)
                    nc.sync.dma_start(y_e[tsl, esl], y_sb[:])

    nc.compile()
    return nc


def make_in_maps(x, wq, bq, wk, bk, wv, bv, wo, bo):
    x = np.asarray(x, np.float32)
    in_maps = []
    for c in range(NCORES):
        b, r = divmod(c, 4)
        in_maps.append({
            "xT": np.ascontiguousarray(x[b].T).astype(BF16NP),
            "wq": np.ascontiguousarray(
                np.asarray(wq)[:, 128 * r:128 * (r + 1)]).astype(BF16NP),
            "wk": np.ascontiguousarray(
                np.asarray(wk)[:, 128 * r:128 * (r + 1)]).astype(BF16NP),
            "wv": np.ascontiguousarray(
                np.asarray(wv)[:, 256 * r:256 * (r + 1)]).astype(BF16NP),
            "wo": np.ascontiguousarray(
                np.asarray(wo)[256 * r:256 * (r + 1), :]).astype(BF16NP),
            "bq": np.ascontiguousarray(
                np.asarray(bq)[128 * r:128 * (r + 1)],
                dtype=np.float32).reshape(-1, 1),
            "bk": np.ascontiguousarray(
                np.asarray(bk)[128 * r:128 * (r + 1)],
                dtype=np.float32).reshape(-1, 1),
            "bv": np.ascontiguousarray(
                np.asarray(bv)[256 * r:256 * (r + 1)],
                dtype=np.float32).reshape(-1, 1),
        })
    return in_maps


def assemble(results, bo):
    y = np.zeros((B, N, D), np.float32)
    for c in range(NCORES):
        y[c // 4] += results[c]["y"]
    return y + np.asarray(bo, np.float32).reshape(1, 1, D)


_NC_CACHE = {}


def run(inputs, trace=False):
    _install_ntff_hook()
    from concourse.bass_utils import run_bass_kernel_spmd
    if "nc" not in _NC_CACHE:
        _NC_CACHE["nc"] = build_nc()
    nc = _NC_CACHE["nc"]
    in_maps = make_in_maps(**inputs)
    res = run_bass_kernel_spmd(nc, in_maps, core_ids=list(range(NCORES)),
                               trace=trace)
    return assemble(res.results, inputs["bo"]), res.exec_time_ns


def kernel(**inputs) -> np.ndarray:
    y, _ = run(inputs, trace=False)
    return y
